# revision 23
# baseline (speedup 1.0000x reference)
"""Trainium2 Bass kernel for the spike-decoder GNN message-passing module.

Math (per batch b, output time tau in [0, T-2], variable v):
  out[b,tau,v] = bias[v]
               + sum_{i,k} w[v,i,k] * x[b,i,tau+k-(K-2)]          (static conv)
               + sum_{e: recv[e]=v} sum_k dw[e,b,tau,k] * x[b,send[e],tau+k-(K-2)]
with w = conv_weight masked at w[i,i,K-1] = 0, x = spikes[...,0] transposed to
[b, nvar, t], and out-of-range x treated as zero.

Sharding: 8 cores = (b in 0..3) x (time half h in 0..1). Each core computes a
1024-wide tau window ([0,1024) or [1023,2047) — one overlapping column keeps
shapes uniform for SPMD).

dyn_weights is the only big tensor. It is stored in DRAM as fp8e4 scaled by
256 (dw ~N(0,0.02); x256 keeps 99.7% of values out of e4m3's subnormal range,
so quantization is ~2^-4 relative — measured 7.3e-3 output rel err vs the
2e-2 gate). The fp8 stream is upcast to bf16 *inside the DMA* (SWDGE cast,
gpsimd queue): HBM reads halve to ~8.4 MB/core and the stream becomes bound
by the SBUF-write fabric (~360-420 GB/s measured) instead of HBM reads
(~318 GB/s measured on the bf16 path). The 1/256 descale is folded into the
gather one-hot (ssend entries = 2^-8, bf16-exact), so device products equal
bf16 products of the fp8-rounded dw exactly.

On-core algorithm (bf16 operands, fp32 PSUM accumulation):
  - xg[e,:] = x[send[e],:]/256 gathered via one-hot matmul on PE (doubles as
    the HAM clock warmup); ScalarE writes TWO copies into one tile (cols
    0:1040 even, cols 1040:2080 shifted by one column) so every DVE window AP
    starts 4B-aligned.
  - products P[e,(k,tau)] = dw_half * sliding-window(xg) on DVE, one
    tensor_tensor per parity half (dw arrives parity-major: [even ks|odd ks])
    with all operands 2-byte, stride-1, 4B-aligned -> DVE 2x_1p mode.
  - k-reduction + recv-scatter + transpose folded into PE: per k, a bf16
    matmul with stationary one-hot recv matrix and moving operand = product
    chunk P[:, mC..mC+C], accumulating into PSUM[v, tau].
  - static conv: 16 bf16 matmuls with stationary wT_k and shifted xpad slices
  - bias: folded into the ScalarE PSUM->SBUF copy (bf16 out; host upcasts)

Queues (FIFO within a queue is the only reliable priority):
  - gpsimd/SWDGE: xpad + ssend FIRST (the gather gates the whole consumer
    chain; on the other queue they starve behind the dw stream at the SDMA
    round-robin), then the dw cast-stream in consumption order.
  - sync/HWDGE: wt/recvT/bias (needed a few us later; they trickle through
    the round-robin) and the two output stores.
Pipeline granularity is a parity HALF-tile (1.05 MB DMA, ~2.2 us DVE, 8 PE
matmuls) so the consumer chain tracks the stream closely; the last tile goes
in quarters to shrink the serial tail. dw tiles use 6 SBUF buffers so the
buffer-reuse (WAR) waits on the Pool engine never pause descriptor emission.
Output is [v, tau] bf16 per core; host upcasts + transposes while assembling.
"""

import numpy as np

B, T, NVAR, K, E = 4, 2048, 128, 16, 512
TAU = T - 1            # 2047
L = 1024               # per-core tau window
NC_COUNT = 8
W_XPAD = L + K         # 1040 (1039 used; even so bf16 tiles stay 4B-aligned)
ETILES = E // 128      # 4
CHUNK = 512            # tau chunk per PSUM bank
NCHUNK = L // CHUNK    # 2
KH = K // 2            # 8 ks per parity
HK = CHUNK * KH        # 4096 product columns per parity half

_PROGRAM = None


def _build_program():
    import concourse.bass as bass
    import concourse.bacc as bacc
    import concourse.mybir as mybir
    import concourse.tile as tile

    f32 = mybir.dt.float32
    bf16 = mybir.dt.bfloat16
    fp8 = mybir.dt.float8e4
    # Bacc (not plain Bass): its compile pipeline runs generate_event_semaphores,
    # which splits multi-semaphore waits — a raw Matmult supports only one
    # sync-wait slot and walrus rejects more ("Too many sync wait commands").
    nc = bacc.Bacc()

    # all bf16 consts packed in one tensor: [xpad | ssend | wt | recvT] —
    # ONE head-of-queue DMA instead of four (fewer descriptor-gens ahead of
    # the dw stream, one completion semaphore). bias (fp32, 512 B) rides
    # sync/HWDGE; it is only needed by the final ScalarE bias-add.
    CW = W_XPAD + E + K * NVAR + ETILES * NVAR
    call_d = nc.declare_dram_parameter("call", [NVAR, CW], bf16, isOutput=False)
    dw_d = nc.declare_dram_parameter("dw", [NCHUNK * E, CHUNK * K], fp8, isOutput=False)
    bias_d = nc.declare_dram_parameter("biasv", [NVAR, 1], f32, isOutput=False)
    y_d = nc.declare_dram_parameter("yT", [NVAR, L], bf16, isOutput=True)

    with tile.TileContext(nc) as tc:
        with (
            tc.tile_pool(name="consts", bufs=1) as consts,
            tc.tile_pool(name="xgp", bufs=1) as xgp,
            tc.tile_pool(name="gpsum", bufs=4, space=bass.MemorySpace.PSUM) as gpsum,
            tc.tile_pool(name="dwp", bufs=5) as dwp,
            tc.tile_pool(name="stage8", bufs=3) as stage8,
            tc.tile_pool(name="prodp", bufs=3) as prodp,
            tc.tile_pool(name="opsum", bufs=2, space=bass.MemorySpace.PSUM) as opsum,
            tc.tile_pool(name="resp", bufs=2) as resp,
        ):
            NT = NCHUNK * ETILES  # 8 dw tiles

            # consts at the HEAD of the gpsimd queue (see docstring)
            call = consts.tile([NVAR, CW], bf16)
            nc.gpsimd.dma_start(call[:], call_d[:])
            xpad = call[:, 0:W_XPAD]
            ssend = call[:, W_XPAD:W_XPAD + E]
            wt = call[:, W_XPAD + E:W_XPAD + E + K * NVAR]
            recvT = call[:, W_XPAD + E + K * NVAR:CW]

            # dw stream: one DMA per parity half (quarters for the last tile)
            # in exact consumption order. Most pieces upcast fp8->bf16 inside
            # the DMA (write-fabric bound, ~2 B/elem written); tiles in RAW8
            # land as raw fp8 (1 B/elem — halves their write-fabric cost) and
            # ScalarE, idle during the stream, does their upcast. RAW8 tiles
            # sit mid-run so ScalarE finishes each well before the DVE needs
            # it; the head and tail tiles stay on the cast path (the head
            # gates the DVE start, the tail would put ScalarE on the final
            # serial chain).
            QUARTERED = {NT - 1}
            RAW8 = {3, 5}
            HALVED = {0} | RAW8

            dwt_tiles = []
            for ti in range(NT):
                dwt = dwp.tile([128, CHUNK * K], bf16, name="dwt", tag="dwt")
                dwt_tiles.append(dwt)

            stage_tiles = {}
            def dw_dma_piece(ti, q, n):
                h2, et = divmod(ti, ETILES)
                r0 = h2 * E + et * 128
                pw = CHUNK * K // n
                if ti in RAW8:
                    st = stage8.tile([128, pw], fp8, name="st", tag="st")
                    nc.gpsimd.dma_start(
                        st[:], dw_d[r0:r0 + 128, q * pw:(q + 1) * pw]
                    )
                    stage_tiles[(ti, q)] = st
                else:
                    nc.gpsimd.dma_start(
                        dwt_tiles[ti][:, q * pw:(q + 1) * pw],
                        dw_d[r0:r0 + 128, q * pw:(q + 1) * pw],
                    )

            for ti in range(NT):
                n = 4 if ti in QUARTERED else (2 if ti in HALVED else 1)
                for q in range(n):
                    dw_dma_piece(ti, q, n)

            # bias (512 B) on sync/HWDGE; needed only by the final bias-add
            biasv = consts.tile([NVAR, 1], f32)
            nc.sync.dma_start(biasv[:], bias_d[:])

            # Gather sender rows (also the PE clock warmup); ssend holds 2^-8
            # one-hots so xg = x/256 (bf16-exact), cancelling the fp8 scale:
            # xgc[et][p, j]      = xpad[send[et*128+p], j] / 256   (cols 0:1040)
            # xgc[et][p, 1040+j] = xpad[send[et*128+p], j+1] / 256 (odd-k copy)
            xgc = []
            for et in range(ETILES):
                xg = xgp.tile([128, 2 * W_XPAD], bf16, name=f"xg{et}", tag=f"xg{et}")
                xrow0 = xg.tensor.shape[-1]
                for j0 in range(0, W_XPAD, CHUNK):
                    jw = min(CHUNK, W_XPAD - j0)
                    gps = gpsum.tile([128, CHUNK], f32, name="gps", tag="gps")
                    grow = gps.tensor.shape[-1]
                    nc.tensor.matmul(
                        gps[:, :jw],
                        ssend[:, et * 128:(et + 1) * 128],
                        xpad[:, j0:j0 + jw],
                        start=True, stop=True,
                    )
                    # both xg copies in ONE ScalarE op via a 3D AP (the
                    # second output row is the 1-shifted duplicate)
                    if j0 == 0:
                        # xg[c]=gps[c], xg[W_XPAD+c]=gps[c+1]: src row stride 1
                        nc.scalar.copy(
                            bass.AP(xg.tensor, 0,
                                    [[xrow0, 128], [W_XPAD, 2], [1, jw - 1]]),
                            bass.AP(gps.tensor, 0,
                                    [[grow, 128], [1, 2], [1, jw - 1]]),
                        )
                        nc.scalar.copy(xg[:, jw - 1:jw], gps[:, jw - 1:jw])
                    else:
                        # xg[j0+c]=gps[c], xg[W_XPAD+j0-1+c]=gps[c]: src row
                        # stride 0 (same element feeds both outputs)
                        nc.scalar.copy(
                            bass.AP(xg.tensor, j0,
                                    [[xrow0, 128], [W_XPAD - 1, 2], [1, jw]]),
                            bass.AP(gps.tensor, 0,
                                    [[grow, 128], [0, 2], [1, jw]]),
                        )
                xgc.append(xg)

            ops_tiles = []
            for h2 in range(NCHUNK):
                o = opsum.tile([128, CHUNK], f32, name=f"ops{h2}", tag=f"ops{h2}")
                ops_tiles.append(o)

            def static_mm(h2, k, start=False):
                t0 = h2 * CHUNK
                nc.tensor.matmul(
                    ops_tiles[h2][:],
                    wt[:, k * NVAR:(k + 1) * NVAR],
                    xpad[:, t0 + k:t0 + k + CHUNK],
                    start=start, stop=False,
                )

            # chunk-0 static conv up front (PE warmup continues while dw streams)
            for k in range(K):
                static_mm(0, k, start=(k == 0))

            # chunk-1 static matmuls fill PE gaps across the first 7 tiles
            fill = [("s", k) for k in range(K)]
            fills_per_group = [3, 3, 2, 2, 2, 2, 2, 0]

            def reduce_mm(h2, et, pt, prow, m, stop):
                rhs = bass.AP(pt.tensor, m * CHUNK, [[prow, 128], [1, CHUNK]])
                nc.tensor.matmul(
                    ops_tiles[h2][:],
                    recvT[:, et * NVAR:(et + 1) * NVAR],
                    rhs,
                    start=False, stop=stop,
                )

            for ti in range(NT):
                h2, et = divmod(ti, ETILES)
                t0 = h2 * CHUNK
                dwt = dwt_tiles[ti]
                drow = dwt.tensor.shape[-1]
                xrow = xgc[et].tensor.shape[-1]
                # dw is parity-major: dwt[e, par*HK + m*CHUNK + tau] holds
                # dw[e, k=2m+par, tau]; window for k is xgc[par*1040 + t0+2m+tau]
                last_of_chunk = et == ETILES - 1
                if ti not in QUARTERED and ti not in HALVED:
                    # whole tile: ONE 4D-AP tensor_tensor covers both parities
                    pt = prodp.tile([128, CHUNK * K], bf16, name="pt", tag="pt")
                    prow = pt.tensor.shape[-1]
                    in0 = bass.AP(dwt.tensor, 0,
                                  [[drow, 128], [HK, 2], [CHUNK, KH], [1, CHUNK]])
                    in1 = bass.AP(xgc[et].tensor, t0,
                                  [[xrow, 128], [W_XPAD, 2], [2, KH], [1, CHUNK]])
                    out4 = bass.AP(pt.tensor, 0,
                                   [[prow, 128], [HK, 2], [CHUNK, KH], [1, CHUNK]])
                    nc.vector.tensor_mul(out4, in0, in1)
                    for m in range(K):
                        reduce_mm(h2, et, pt, prow, m,
                                  stop=(last_of_chunk and m == K - 1))
                else:
                    npieces = 4 if ti in QUARTERED else 2
                    # piece q covers ks of parity par = q*2//n, window rows
                    # m = hf*(KH//(n//2)) ..; products in a half-size tile
                    mh = KH // (npieces // 2)  # k-rows per piece
                    pw = CHUNK * K // npieces
                    for q in range(npieces):
                        par, hf = divmod(q, npieces // 2)
                        if (ti, q) in stage_tiles:
                            # raw-fp8 piece: ScalarE upcast into the bf16 tile
                            nc.scalar.copy(
                                dwt_tiles[ti][:, q * pw:(q + 1) * pw],
                                stage_tiles[(ti, q)][:],
                            )
                        ptq = prodp.tile([128, HK], bf16, name="ptq", tag="ptq")
                        prow = ptq.tensor.shape[-1]
                        in0 = bass.AP(dwt.tensor, q * pw,
                                      [[drow, 128], [CHUNK, mh], [1, CHUNK]])
                        in1 = bass.AP(xgc[et].tensor,
                                      par * W_XPAD + t0 + 2 * (hf * mh),
                                      [[xrow, 128], [2, mh], [1, CHUNK]])
                        out3 = bass.AP(ptq.tensor, 0,
                                       [[prow, 128], [CHUNK, mh], [1, CHUNK]])
                        nc.vector.tensor_mul(out3, in0, in1)
                        for m in range(mh):
                            reduce_mm(h2, et, ptq, prow, m,
                                      stop=(last_of_chunk and q == npieces - 1
                                            and m == mh - 1))
                for _ in range(fills_per_group[ti]):
                    _, k = fill.pop(0)
                    static_mm(1, k, start=(k == 0))
                if et == ETILES - 1:
                    res = resp.tile([128, CHUNK], bf16, name="res", tag="res")
                    # PSUM -> SBUF copy with the conv bias added (bf16 store;
                    # host upcasts — output tolerance is 2e-2)
                    nc.scalar.add(res[:], ops_tiles[h2][:], biasv[:, 0:1])
                    nc.sync.dma_start(y_d[:, t0:t0 + CHUNK], res[:])

    nc.compile()
    return nc


def _get_program():
    global _PROGRAM
    if _PROGRAM is None:
        _PROGRAM = _build_program()
    return _PROGRAM


# k order inside a parity-major dw row: evens then odds
_KORDER = list(range(0, K, 2)) + list(range(1, K, 2))


def _host_prep(spikes, conv_weight, conv_bias, dyn_weights, edge_send, edge_recv):
    import ml_dtypes
    bf = ml_dtypes.bfloat16
    f8 = ml_dtypes.float8_e4m3

    spikes = np.asarray(spikes, dtype=np.float32)
    conv_weight = np.asarray(conv_weight, dtype=np.float32)
    conv_bias = np.asarray(conv_bias, dtype=np.float32)
    dyn_weights = np.asarray(dyn_weights, dtype=np.float32)
    edge_send = np.asarray(edge_send, dtype=np.int64)
    edge_recv = np.asarray(edge_recv, dtype=np.int64)

    x = np.ascontiguousarray(spikes[..., 0].transpose(0, 2, 1))  # [B, NVAR, T]
    # one bulk fp32->fp8 pass; x256 so ~N(0,5) lands in e4m3's normal range
    dyn8 = (dyn_weights * 256.0).astype(f8)

    ssend = np.zeros((NVAR, E), bf)
    ssend[edge_send, np.arange(E)] = 1.0 / 256.0  # descale folded into gather

    recvT = np.zeros((128, ETILES * NVAR), bf)
    for et in range(ETILES):
        rr = edge_recv[et * 128:(et + 1) * 128]
        recvT[np.arange(128), et * NVAR + rr] = 1.0

    w = conv_weight.copy()
    w[np.arange(NVAR), np.arange(NVAR), K - 1] = 0.0
    wt = np.ascontiguousarray(w.transpose(1, 2, 0)).reshape(NVAR, K * NVAR).astype(bf)

    biasv = conv_bias.reshape(NVAR, 1).astype(np.float32)

    CW = W_XPAD + E + K * NVAR + ETILES * NVAR
    in_maps = []
    for core in range(NC_COUNT):
        b, h = divmod(core, 2)
        tau0 = 0 if h == 0 else TAU - L  # 0 or 1023
        call = np.zeros((NVAR, CW), bf)
        lo = tau0 - (K - 2)  # first x column needed
        src_lo = max(lo, 0)
        call[:, src_lo - lo:W_XPAD - 1] = x[b, :, src_lo:tau0 + L + 1]
        call[:, W_XPAD:W_XPAD + E] = ssend
        call[:, W_XPAD + E:W_XPAD + E + K * NVAR] = wt
        call[:, W_XPAD + E + K * NVAR:CW] = recvT
        a = dyn8[:, b, tau0:tau0 + L, :]                 # [E, L, K]
        a = a.reshape(E, NCHUNK, CHUNK, K)               # [E, h2, tau, k]
        a = a.transpose(1, 0, 3, 2)[:, :, _KORDER, :]    # [h2, E, kpar, tau]
        dw = np.ascontiguousarray(a).reshape(NCHUNK * E, CHUNK * K)
        in_maps.append({
            "call": call,
            "dw": dw,
            "biasv": biasv,
        })
    return in_maps


def _assemble(results):
    out = np.empty((B, TAU, NVAR, 1), np.float32)
    for core in range(NC_COUNT):
        b, h = divmod(core, 2)
        yT = results[core]["yT"].astype(np.float32)  # [NVAR, L] bf16 -> fp32
        if h == 0:
            out[b, 0:L, :, 0] = yT.T
        else:
            out[b, L:TAU, :, 0] = yT[:, 1:L].T
    return out


def run_on_hw(in_maps, trace=False, **kwargs):
    from concourse.bass_utils import run_bass_kernel_spmd

    nc = _get_program()
    return run_bass_kernel_spmd(
        nc, in_maps, core_ids=list(range(NC_COUNT)), trace=trace, **kwargs
    )


def kernel(spikes, conv_weight, conv_bias, dyn_weights, edge_send, edge_recv):
    in_maps = _host_prep(
        spikes, conv_weight, conv_bias, dyn_weights, edge_send, edge_recv
    )
    res = run_on_hw(in_maps)
    return _assemble(res.results)


# revision 25
# speedup vs baseline: 1.1121x; 1.1121x over previous
"""Trainium2 Bass kernel for the spike-decoder GNN message-passing module.

Math (per batch b, output time tau in [0, T-2], variable v):
  out[b,tau,v] = bias[v]
               + sum_{i,k} w[v,i,k] * x[b,i,tau+k-(K-2)]          (static conv)
               + sum_{e: recv[e]=v} sum_k dw[e,b,tau,k] * x[b,send[e],tau+k-(K-2)]
with w = conv_weight masked at w[i,i,K-1] = 0, x = spikes[...,0] transposed to
[b, nvar, t], and out-of-range x treated as zero.

Sharding: 8 cores = (b in 0..3) x (time half h in 0..1). Each core computes a
1024-wide tau window ([0,1024) or [1023,2047) — one overlapping column keeps
shapes uniform for SPMD).

dyn_weights is the only big tensor. It is stored in DRAM as fp8e4 scaled by
256 (dw ~N(0,0.02); x256 keeps 99.7% of values out of e4m3's subnormal range,
so quantization is ~2^-4 relative — measured 7.3e-3 output rel err vs the
2e-2 gate). The fp8 stream is upcast to bf16 *inside the DMA* (SWDGE cast,
gpsimd queue): HBM reads halve to ~8.4 MB/core and the stream becomes bound
by the SBUF-write fabric (~360-420 GB/s measured) instead of HBM reads
(~318 GB/s measured on the bf16 path). The 1/256 descale is folded into the
gather one-hot (ssend entries = 2^-8, bf16-exact), so device products equal
bf16 products of the fp8-rounded dw exactly.

On-core algorithm (bf16 operands, fp32 PSUM accumulation):
  - xg[e,:] = x[send[e],:]/256 gathered via one-hot matmul on PE (doubles as
    the HAM clock warmup); ScalarE writes TWO copies into one tile (cols
    0:1040 even, cols 1040:2080 shifted by one column) so every DVE window AP
    starts 4B-aligned.
  - products P[e,(k,tau)] = dw_half * sliding-window(xg) on DVE, one
    tensor_tensor per parity half (dw arrives parity-major: [even ks|odd ks])
    with all operands 2-byte, stride-1, 4B-aligned -> DVE 2x_1p mode.
  - k-reduction + recv-scatter + transpose folded into PE: per k, a bf16
    matmul with stationary one-hot recv matrix and moving operand = product
    chunk P[:, mC..mC+C], accumulating into PSUM[v, tau].
  - static conv: 16 bf16 matmuls with stationary wT_k and shifted xpad slices
  - bias: folded into the ScalarE PSUM->SBUF copy (bf16 out; host upcasts)

Queues (FIFO within a queue is the only reliable priority):
  - gpsimd/SWDGE: xpad + ssend FIRST (the gather gates the whole consumer
    chain; on the other queue they starve behind the dw stream at the SDMA
    round-robin), then the dw cast-stream in consumption order.
  - sync/HWDGE: wt/recvT/bias (needed a few us later; they trickle through
    the round-robin) and the two output stores.
Pipeline granularity is a parity HALF-tile (1.05 MB DMA, ~2.2 us DVE, 8 PE
matmuls) so the consumer chain tracks the stream closely; the last tile goes
in quarters to shrink the serial tail. dw tiles use 6 SBUF buffers so the
buffer-reuse (WAR) waits on the Pool engine never pause descriptor emission.
Output is [v, tau] bf16 per core; host upcasts + transposes while assembling.
"""

import numpy as np

B, T, NVAR, K, E = 4, 2048, 128, 16, 512
TAU = T - 1            # 2047
L = 1024               # per-core tau window
NC_COUNT = 8
W_XPAD = L + K         # 1040 (1039 used; even so bf16 tiles stay 4B-aligned)
ETILES = E // 128      # 4
CHUNK = 512            # tau chunk per PSUM bank
NCHUNK = L // CHUNK    # 2
KH = K // 2            # 8 ks per parity
HK = CHUNK * KH        # 4096 product columns per parity half

_PROGRAM = None


def _build_program():
    import concourse.bass as bass
    import concourse.bacc as bacc
    import concourse.mybir as mybir
    import concourse.tile as tile

    f32 = mybir.dt.float32
    bf16 = mybir.dt.bfloat16
    fp8 = mybir.dt.float8e4
    # Bacc (not plain Bass): its compile pipeline runs generate_event_semaphores,
    # which splits multi-semaphore waits — a raw Matmult supports only one
    # sync-wait slot and walrus rejects more ("Too many sync wait commands").
    nc = bacc.Bacc()

    # all bf16 consts packed in one tensor: [xpad | ssend | wt | recvT] —
    # ONE head-of-queue DMA instead of four (fewer descriptor-gens ahead of
    # the dw stream, one completion semaphore). bias (fp32, 512 B) rides
    # sync/HWDGE; it is only needed by the final ScalarE bias-add.
    CW = W_XPAD + E + K * NVAR + ETILES * NVAR
    call_d = nc.declare_dram_parameter("call", [NVAR, CW], bf16, isOutput=False)
    dw_d = nc.declare_dram_parameter("dw", [NCHUNK * E, CHUNK * K], fp8, isOutput=False)
    bias_d = nc.declare_dram_parameter("biasv", [NVAR, 1], f32, isOutput=False)
    y_d = nc.declare_dram_parameter("yT", [NVAR, L], bf16, isOutput=True)

    with tile.TileContext(nc) as tc:
        with (
            tc.tile_pool(name="consts", bufs=1) as consts,
            tc.tile_pool(name="xgp", bufs=1) as xgp,
            tc.tile_pool(name="gpsum", bufs=4, space=bass.MemorySpace.PSUM) as gpsum,
            tc.tile_pool(name="dwp", bufs=6) as dwp,
            tc.tile_pool(name="stage8", bufs=3) as stage8,
            tc.tile_pool(name="prodp", bufs=5) as prodp,
            tc.tile_pool(name="opsum", bufs=2, space=bass.MemorySpace.PSUM) as opsum,
            tc.tile_pool(name="resp", bufs=2) as resp,
        ):
            NT = NCHUNK * ETILES  # 8 dw tiles

            # consts at the HEAD of the gpsimd queue (see docstring)
            call = consts.tile([NVAR, CW], bf16)
            nc.gpsimd.dma_start(call[:], call_d[:])
            xpad = call[:, 0:W_XPAD]
            ssend = call[:, W_XPAD:W_XPAD + E]
            wt = call[:, W_XPAD + E:W_XPAD + E + K * NVAR]
            recvT = call[:, W_XPAD + E + K * NVAR:CW]

            # dw stream: one DMA per parity half (quarters for the last tile)
            # in exact consumption order. Most pieces upcast fp8->bf16 inside
            # the DMA (write-fabric bound, ~2 B/elem written); tiles in RAW8
            # land as raw fp8 (1 B/elem — halves their write-fabric cost) and
            # ScalarE, idle during the stream, does their upcast. RAW8 tiles
            # sit mid-run so ScalarE finishes each well before the DVE needs
            # it; the head and tail tiles stay on the cast path (the head
            # gates the DVE start, the tail would put ScalarE on the final
            # serial chain).
            QUARTERED = {NT - 1}
            RAW8 = set()
            HALVED = set(range(NT - 1))

            dwt_tiles = []
            for ti in range(NT):
                dwt = dwp.tile([128, CHUNK * K], bf16, name="dwt", tag="dwt")
                dwt_tiles.append(dwt)

            stage_tiles = {}
            def dw_dma_piece(ti, q, n):
                h2, et = divmod(ti, ETILES)
                r0 = h2 * E + et * 128
                pw = CHUNK * K // n
                if ti in RAW8:
                    st = stage8.tile([128, pw], fp8, name="st", tag="st")
                    nc.gpsimd.dma_start(
                        st[:], dw_d[r0:r0 + 128, q * pw:(q + 1) * pw]
                    )
                    stage_tiles[(ti, q)] = st
                else:
                    nc.gpsimd.dma_start(
                        dwt_tiles[ti][:, q * pw:(q + 1) * pw],
                        dw_d[r0:r0 + 128, q * pw:(q + 1) * pw],
                    )

            for ti in range(NT):
                n = 4 if ti in QUARTERED else (2 if ti in HALVED else 1)
                for q in range(n):
                    dw_dma_piece(ti, q, n)

            # bias (512 B) on sync/HWDGE; needed only by the final bias-add
            biasv = consts.tile([NVAR, 1], f32)
            nc.sync.dma_start(biasv[:], bias_d[:])

            # Gather sender rows (also the PE clock warmup); ssend holds 2^-8
            # one-hots so xg = x/256 (bf16-exact), cancelling the fp8 scale:
            # xgc[et][p, j]      = xpad[send[et*128+p], j] / 256   (cols 0:1040)
            # xgc[et][p, 1040+j] = xpad[send[et*128+p], j+1] / 256 (odd-k copy)
            xgc = []
            for et in range(ETILES):
                xg = xgp.tile([128, 2 * W_XPAD], bf16, name=f"xg{et}", tag=f"xg{et}")
                xrow0 = xg.tensor.shape[-1]
                for j0 in range(0, W_XPAD, CHUNK):
                    jw = min(CHUNK, W_XPAD - j0)
                    gps = gpsum.tile([128, CHUNK], f32, name="gps", tag="gps")
                    grow = gps.tensor.shape[-1]
                    nc.tensor.matmul(
                        gps[:, :jw],
                        ssend[:, et * 128:(et + 1) * 128],
                        xpad[:, j0:j0 + jw],
                        start=True, stop=True,
                    )
                    # both xg copies in ONE ScalarE op via a 3D AP (the
                    # second output row is the 1-shifted duplicate)
                    if j0 == 0:
                        # xg[c]=gps[c], xg[W_XPAD+c]=gps[c+1]: src row stride 1
                        nc.scalar.copy(
                            bass.AP(xg.tensor, 0,
                                    [[xrow0, 128], [W_XPAD, 2], [1, jw - 1]]),
                            bass.AP(gps.tensor, 0,
                                    [[grow, 128], [1, 2], [1, jw - 1]]),
                        )
                        nc.scalar.copy(xg[:, jw - 1:jw], gps[:, jw - 1:jw])
                    else:
                        # xg[j0+c]=gps[c], xg[W_XPAD+j0-1+c]=gps[c]: src row
                        # stride 0 (same element feeds both outputs)
                        nc.scalar.copy(
                            bass.AP(xg.tensor, j0,
                                    [[xrow0, 128], [W_XPAD - 1, 2], [1, jw]]),
                            bass.AP(gps.tensor, 0,
                                    [[grow, 128], [0, 2], [1, jw]]),
                        )
                xgc.append(xg)

            ops_tiles = []
            for h2 in range(NCHUNK):
                o = opsum.tile([128, CHUNK], f32, name=f"ops{h2}", tag=f"ops{h2}")
                ops_tiles.append(o)

            def static_mm(h2, k, start=False):
                t0 = h2 * CHUNK
                nc.tensor.matmul(
                    ops_tiles[h2][:],
                    wt[:, k * NVAR:(k + 1) * NVAR],
                    xpad[:, t0 + k:t0 + k + CHUNK],
                    start=start, stop=False,
                )

            # chunk-0 static conv up front (PE warmup continues while dw streams)
            for k in range(K):
                static_mm(0, k, start=(k == 0))

            # chunk-1 static matmuls fill PE gaps across the first 7 tiles
            fill = [("s", k) for k in range(K)]
            fills_per_group = [3, 3, 2, 2, 2, 2, 2, 0]

            def reduce_mm(h2, et, pt, prow, m, stop):
                rhs = bass.AP(pt.tensor, m * CHUNK, [[prow, 128], [1, CHUNK]])
                nc.tensor.matmul(
                    ops_tiles[h2][:],
                    recvT[:, et * NVAR:(et + 1) * NVAR],
                    rhs,
                    start=False, stop=stop,
                )

            for ti in range(NT):
                h2, et = divmod(ti, ETILES)
                t0 = h2 * CHUNK
                dwt = dwt_tiles[ti]
                drow = dwt.tensor.shape[-1]
                xrow = xgc[et].tensor.shape[-1]
                # dw is parity-major: dwt[e, par*HK + m*CHUNK + tau] holds
                # dw[e, k=2m+par, tau]; window for k is xgc[par*1040 + t0+2m+tau]
                last_of_chunk = et == ETILES - 1
                if ti not in QUARTERED and ti not in HALVED:
                    # whole tile: ONE 4D-AP tensor_tensor covers both parities
                    pt = prodp.tile([128, CHUNK * K], bf16, name="pt", tag="pt")
                    prow = pt.tensor.shape[-1]
                    in0 = bass.AP(dwt.tensor, 0,
                                  [[drow, 128], [HK, 2], [CHUNK, KH], [1, CHUNK]])
                    in1 = bass.AP(xgc[et].tensor, t0,
                                  [[xrow, 128], [W_XPAD, 2], [2, KH], [1, CHUNK]])
                    out4 = bass.AP(pt.tensor, 0,
                                   [[prow, 128], [HK, 2], [CHUNK, KH], [1, CHUNK]])
                    nc.vector.tensor_mul(out4, in0, in1)
                    for m in range(K):
                        reduce_mm(h2, et, pt, prow, m,
                                  stop=(last_of_chunk and m == K - 1))
                else:
                    npieces = 4 if ti in QUARTERED else 2
                    # piece q covers ks of parity par = q*2//n, window rows
                    # m = hf*(KH//(n//2)) ..; products in a half-size tile
                    mh = KH // (npieces // 2)  # k-rows per piece
                    pw = CHUNK * K // npieces
                    for q in range(npieces):
                        par, hf = divmod(q, npieces // 2)
                        if (ti, q) in stage_tiles:
                            # raw-fp8 piece: ScalarE upcast into the bf16 tile
                            nc.scalar.copy(
                                dwt_tiles[ti][:, q * pw:(q + 1) * pw],
                                stage_tiles[(ti, q)][:],
                            )
                        ptq = prodp.tile([128, HK], bf16, name="ptq", tag="ptq")
                        prow = ptq.tensor.shape[-1]
                        in0 = bass.AP(dwt.tensor, q * pw,
                                      [[drow, 128], [CHUNK, mh], [1, CHUNK]])
                        in1 = bass.AP(xgc[et].tensor,
                                      par * W_XPAD + t0 + 2 * (hf * mh),
                                      [[xrow, 128], [2, mh], [1, CHUNK]])
                        out3 = bass.AP(ptq.tensor, 0,
                                       [[prow, 128], [CHUNK, mh], [1, CHUNK]])
                        nc.vector.tensor_mul(out3, in0, in1)
                        for m in range(mh):
                            reduce_mm(h2, et, ptq, prow, m,
                                      stop=(last_of_chunk and q == npieces - 1
                                            and m == mh - 1))
                for _ in range(fills_per_group[ti]):
                    _, k = fill.pop(0)
                    static_mm(1, k, start=(k == 0))
                if et == ETILES - 1:
                    res = resp.tile([128, CHUNK], bf16, name="res", tag="res")
                    # PSUM -> SBUF copy with the conv bias added (bf16 store;
                    # host upcasts — output tolerance is 2e-2)
                    nc.scalar.add(res[:], ops_tiles[h2][:], biasv[:, 0:1])
                    nc.sync.dma_start(y_d[:, t0:t0 + CHUNK], res[:])

    nc.compile()
    return nc


def _get_program():
    global _PROGRAM
    if _PROGRAM is None:
        _PROGRAM = _build_program()
    return _PROGRAM


# k order inside a parity-major dw row: evens then odds
_KORDER = list(range(0, K, 2)) + list(range(1, K, 2))


def _host_prep(spikes, conv_weight, conv_bias, dyn_weights, edge_send, edge_recv):
    import ml_dtypes
    bf = ml_dtypes.bfloat16
    f8 = ml_dtypes.float8_e4m3

    spikes = np.asarray(spikes, dtype=np.float32)
    conv_weight = np.asarray(conv_weight, dtype=np.float32)
    conv_bias = np.asarray(conv_bias, dtype=np.float32)
    dyn_weights = np.asarray(dyn_weights, dtype=np.float32)
    edge_send = np.asarray(edge_send, dtype=np.int64)
    edge_recv = np.asarray(edge_recv, dtype=np.int64)

    x = np.ascontiguousarray(spikes[..., 0].transpose(0, 2, 1))  # [B, NVAR, T]
    # one bulk fp32->fp8 pass; x256 so ~N(0,5) lands in e4m3's normal range
    dyn8 = (dyn_weights * 256.0).astype(f8)

    ssend = np.zeros((NVAR, E), bf)
    ssend[edge_send, np.arange(E)] = 1.0 / 256.0  # descale folded into gather

    recvT = np.zeros((128, ETILES * NVAR), bf)
    for et in range(ETILES):
        rr = edge_recv[et * 128:(et + 1) * 128]
        recvT[np.arange(128), et * NVAR + rr] = 1.0

    w = conv_weight.copy()
    w[np.arange(NVAR), np.arange(NVAR), K - 1] = 0.0
    wt = np.ascontiguousarray(w.transpose(1, 2, 0)).reshape(NVAR, K * NVAR).astype(bf)

    biasv = conv_bias.reshape(NVAR, 1).astype(np.float32)

    CW = W_XPAD + E + K * NVAR + ETILES * NVAR
    in_maps = []
    for core in range(NC_COUNT):
        b, h = divmod(core, 2)
        tau0 = 0 if h == 0 else TAU - L  # 0 or 1023
        call = np.zeros((NVAR, CW), bf)
        lo = tau0 - (K - 2)  # first x column needed
        src_lo = max(lo, 0)
        call[:, src_lo - lo:W_XPAD - 1] = x[b, :, src_lo:tau0 + L + 1]
        call[:, W_XPAD:W_XPAD + E] = ssend
        call[:, W_XPAD + E:W_XPAD + E + K * NVAR] = wt
        call[:, W_XPAD + E + K * NVAR:CW] = recvT
        a = dyn8[:, b, tau0:tau0 + L, :]                 # [E, L, K]
        a = a.reshape(E, NCHUNK, CHUNK, K)               # [E, h2, tau, k]
        a = a.transpose(1, 0, 3, 2)[:, :, _KORDER, :]    # [h2, E, kpar, tau]
        dw = np.ascontiguousarray(a).reshape(NCHUNK * E, CHUNK * K)
        in_maps.append({
            "call": call,
            "dw": dw,
            "biasv": biasv,
        })
    return in_maps


def _assemble(results):
    out = np.empty((B, TAU, NVAR, 1), np.float32)
    for core in range(NC_COUNT):
        b, h = divmod(core, 2)
        yT = results[core]["yT"].astype(np.float32)  # [NVAR, L] bf16 -> fp32
        if h == 0:
            out[b, 0:L, :, 0] = yT.T
        else:
            out[b, L:TAU, :, 0] = yT[:, 1:L].T
    return out


def run_on_hw(in_maps, trace=False, **kwargs):
    from concourse.bass_utils import run_bass_kernel_spmd

    nc = _get_program()
    return run_bass_kernel_spmd(
        nc, in_maps, core_ids=list(range(NC_COUNT)), trace=trace, **kwargs
    )


def kernel(spikes, conv_weight, conv_bias, dyn_weights, edge_send, edge_recv):
    in_maps = _host_prep(
        spikes, conv_weight, conv_bias, dyn_weights, edge_send, edge_recv
    )
    res = run_on_hw(in_maps)
    return _assemble(res.results)
